# revision 33
# baseline (speedup 1.0000x reference)
"""Exact top-k (k=32) attention on 8 Trainium2 NeuronCores.

Head-parallel sharding: core c computes (batch 0, head c) and (batch 1,
head c).  Per-batch key-length truncation: only SC = ceil(kl/128) chunks
of 128 keys are ever touched (the rest can never enter the top-32), so
each core's two head-slots run with different (smaller) S.

Per head, per core:
  Phase 1 (selection): forward scores F[q, s] via a 2-pass bf16-split
    matmul (hi*hi + partial lo*lo in pass A; hi*lo + lo*hi in pass BC;
    ~1e-5 accurate).  Hierarchical exact top-32: per-128-chunk top-8 via
    one DVE max8 each, reading PSUM directly (level 1), then top-32 of
    the <=128 candidates via 4x max8 + 3x match_replace (level 2).
    Cut value t_minus = t - |t|*2^-16 - 1e-37, strictly inside
    (s_33 + eps, s_32); bf16 triple-split of -t_minus is staged into
    rows 65..67 of the qa operand.
  Phase 2 (apply): transposed scores minus t_minus computed by the same
    augmented matmul pair (extra contraction rows carry the mask and -t
    split), giving d'[s, q] = F^T - t_minus in PSUM.  Then
      g = Exp(temp*d')      (ScalarE, bf16)
      S = Sign(d')          (ScalarE, bf16, {-1,+1})
      A' = max(g-1, 0)      (DVE, bf16 2x mode)
    and the weights W = A' + 0.5*(S+1) are applied via two matmul chains
    accumulated into ONE [66, 512] PSUM tile:
      av = va_a^T A' + va_s^T S,   va_a = [V, 1, 0], va_s = 0.5*[V, 1, 1]
    plus a host-precomputed column bias hs = [0.5*sum V, 0.5*se, 0.5*se]
    added during the PSUM->SBUF copy.  Row 64 = Z, row 65 = the EXACT
    selection count (integer); host recomputes rows with count != 32
    (chunk overflow in level 1 or cut-boundary flips; ~1-3% of rows).
  Emission interleaves phase 1 of unit k+1 tile-by-tile with phase 2 of
  unit k chunk-range-by-chunk-range so every engine stays fed.
"""

import numpy as np
import ml_dtypes

N, L, S, H, E, D = 2, 2048, 2048, 8, 64, 64
TOPK = 32
TEMP = 1.0 / np.sqrt(E)
HEADS_PER_CORE = 2
N_CORES = 8
LT = 16          # L tiles of 128
QB = 4           # q groups of 512
NEG = -1e30
NLO = 60         # e-rows of the lo*lo partial correction in pass A

_bf16 = ml_dtypes.bfloat16


def _build_bass(sc):
    """sc: tuple (SC0, SC1) chunk counts (128 keys each) per head-slot."""
    import concourse.mybir as mybir
    from concourse import bacc
    from concourse.tile import TileContext
    from concourse.masks import make_identity

    f32 = mybir.dt.float32
    bf16 = mybir.dt.bfloat16
    OP = mybir.AluOpType
    AF = mybir.ActivationFunctionType

    nc = bacc.Bacc()
    HPC = HEADS_PER_CORE

    qa_d, ka_d, qbc_d, kbc_d, va_d, hs_d, out_d, cnt_d = ([] for _ in
                                                          range(8))
    for hh in range(HPC):
        se = sc[hh] * 128
        qa_d.append(nc.declare_dram_parameter(f"qa{hh}", [128, L], bf16,
                                              isOutput=False))
        ka_d.append(nc.declare_dram_parameter(f"ka{hh}", [128, se], bf16,
                                              isOutput=False))
        qbc_d.append(nc.declare_dram_parameter(f"qbc{hh}", [128, L], bf16,
                                               isOutput=False))
        kbc_d.append(nc.declare_dram_parameter(f"kbc{hh}", [128, se], bf16,
                                               isOutput=False))
        va_d.append(nc.declare_dram_parameter(f"va{hh}", [2, sc[hh], 128,
                                                          D + 2],
                                              bf16, isOutput=False))
        hs_d.append(nc.declare_dram_parameter(f"hs{hh}", [D + 2, 1], f32,
                                              isOutput=False))
        out_d.append(nc.declare_dram_parameter(f"out{hh}", [L, D], f32,
                                               isOutput=True))
        cnt_d.append(nc.declare_dram_parameter(f"cnt{hh}", [QB, 128, 4],
                                               f32, isOutput=True))

    from contextlib import ExitStack
    with TileContext(nc) as tc, ExitStack() as ctx:
        consts = ctx.enter_context(tc.tile_pool(name="consts", bufs=1))
        inpool = ctx.enter_context(tc.tile_pool(name="inputs", bufs=1))
        mpool = ctx.enter_context(tc.tile_pool(name="mbuf", bufs=2))
        small = ctx.enter_context(tc.tile_pool(name="small", bufs=2))
        gs_pool = ctx.enter_context(tc.tile_pool(name="gs", bufs=4))
        opool = ctx.enter_context(tc.tile_pool(name="outbuf", bufs=3))
        ps_f = ctx.enter_context(tc.tile_pool(name="ps_fwd", bufs=2,
                                              space="PSUM"))
        ps_t = ctx.enter_context(tc.tile_pool(name="ps_t", bufs=3,
                                              space="PSUM"))
        ps_av = ctx.enter_context(tc.tile_pool(name="ps_av", bufs=1,
                                               space="PSUM"))
        ps_x = ctx.enter_context(tc.tile_pool(name="ps_x", bufs=1,
                                              space="PSUM"))

        ident = consts.tile([128, 128], bf16)
        make_identity(nc, ident)
        ident32 = consts.tile([128, 128], f32)
        make_identity(nc, ident32)

        # ---- load inputs (head-0 score operands first) ----
        qa, ka, qbc, kbc, va, hs = [], [], [], [], [], []
        for hh in range(HPC):
            se = sc[hh] * 128
            t = inpool.tile([128, se], bf16, tag=f"ka{hh}", name=f"ka{hh}")
            nc.sync.dma_start(t, ka_d[hh][:, :])
            ka.append(t)
            t = inpool.tile([128, L], bf16, tag=f"qa{hh}", name=f"qa{hh}")
            nc.sync.dma_start(t, qa_d[hh][:, :])
            qa.append(t)
            t = inpool.tile([128, se], bf16, tag=f"kbc{hh}", name=f"kbc{hh}")
            nc.sync.dma_start(t, kbc_d[hh][:, :])
            kbc.append(t)
            t = inpool.tile([128, L], bf16, tag=f"qbc{hh}", name=f"qbc{hh}")
            nc.sync.dma_start(t, qbc_d[hh][:, :])
            qbc.append(t)
        for hh in range(HPC):
            t = inpool.tile([128, 2, sc[hh], D + 2], bf16, tag=f"va{hh}",
                            name=f"va{hh}")
            nc.sync.dma_start(t, va_d[hh][:, :, :, :].rearrange(
                "a c p d -> p a c d"))
            va.append(t)
            t = inpool.tile([D + 2, 1], f32, tag=f"hs{hh}", name=f"hs{hh}")
            nc.sync.dma_start(t, hs_d[hh][:, :])
            hs.append(t)

        # per-group phase-1 state: dg (t32 columns), tcols
        p1s = {}
        p2s = {}

        def p1_tile(hh, g, i):
            """fwd scores + level1 + level2 for tile 4g+i of head hh."""
            SC = sc[hh]
            se = SC * 128
            if i == 0:
                p1s[(hh, g)] = {
                    "tcols": small.tile([128, 12], bf16, tag="tcols",
                                        name="tcols"),
                    "dg": opool.tile([128, 4], f32, tag="dg", name="dg"),
                }
            st = p1s[(hh, g)]
            lt = 4 * g + i
            lhsA = qa[hh][:, lt * 128:(lt + 1) * 128]
            lhsBC = qbc[hh][:, lt * 128:(lt + 1) * 128]
            M = mpool.tile([128, SC * 8], f32, tag="M", name="M")
            nblk = (se + 511) // 512
            for b in range(nblk):
                cw = min(512, se - 512 * b)
                cs = slice(512 * b, 512 * b + cw)
                pf = ps_f.tile([128, 512], f32, tag="fwd", name="fwd")
                nc.tensor.matmul(pf[:, 0:cw], lhsA, ka[hh][:, cs],
                                 start=True, stop=False)
                nc.tensor.matmul(pf[:, 0:cw], lhsBC, kbc[hh][:, cs],
                                 start=False, stop=True)
                for c in range(cw // 128):
                    cc = 4 * b + c
                    nc.vector.max(out=M[:, 8 * cc:8 * cc + 8],
                                  in_=pf[:, 128 * c:128 * c + 128])
            m32 = small.tile([128, 32], f32, tag="m32", name="m32")
            for r in range(4):
                nc.vector.max(out=m32[:, 8 * r:8 * r + 8], in_=M)
                if r < 3:
                    nc.vector.match_replace(
                        out=M, in_to_replace=m32[:, 8 * r:8 * r + 8],
                        in_values=M, imm_value=NEG)
            nc.vector.tensor_copy(st["dg"][:, i:i + 1], m32[:, 31:32])

        def p1_end(hh, g):
            """batched t-ops + staging of the -t_minus bf16 triple-split."""
            st = p1s.pop((hh, g))
            tcols, dg = st["tcols"], st["dg"]
            aco = small.tile([128, 12], f32, tag="aco", name="aco")
            nc.scalar.activation(aco[:, 0:4], dg, AF.Abs,
                                 scale=float(2.0 ** -16))
            nc.vector.scalar_tensor_tensor(
                out=aco[:, 4:8], in0=aco[:, 0:4], scalar=1e-37,
                in1=dg, op0=OP.add, op1=OP.subtract)
            nc.vector.tensor_copy(tcols[:, 0:4], aco[:, 4:8])
            nc.vector.tensor_tensor(out=aco[:, 8:12], in0=aco[:, 4:8],
                                    in1=tcols[:, 0:4], op=OP.subtract)
            nc.vector.tensor_copy(tcols[:, 4:8], aco[:, 8:12])
            nc.vector.tensor_tensor(out=aco[:, 0:4], in0=aco[:, 8:12],
                                    in1=tcols[:, 4:8], op=OP.subtract)
            nc.vector.tensor_copy(tcols[:, 8:12], aco[:, 0:4])
            pt = ps_x.tile([128, 128], bf16, tag="tposeb", name="tposeb")
            nc.tensor.transpose(pt[0:12, :], tcols, ident)
            stage = small.tile([12, 128], bf16, tag="stage12", name="stage12")
            nc.scalar.copy(out=stage, in_=pt[0:12, :])
            for j in range(3):
                nc.sync.dma_start(
                    qa[hh][65 + j:66 + j, g * 512:(g + 1) * 512].rearrange(
                        "p (t q) -> p t q", t=4),
                    stage[4 * j:4 * (j + 1), :])

        def p2_chunks(hh, g, c0, c1):
            """d' + masks + AV for chunks [c0, c1) of unit (hh, g)."""
            SC = sc[hh]
            qs = slice(g * 512, (g + 1) * 512)
            if c0 == 0:
                p2s[(hh, g)] = {
                    "av": ps_av.tile([D + 2, 512], f32, tag="av", name="av"),
                    "ap": [None] * SC, "sg": [None] * SC,
                }
            st = p2s[(hh, g)]

            def emit_av(c, stop):
                nc.tensor.matmul(st["av"], va[hh][:, 0, c, :], st["ap"][c],
                                 start=(c == 0), stop=False)
                nc.tensor.matmul(st["av"], va[hh][:, 1, c, :], st["sg"][c],
                                 start=False, stop=stop)

            for c in range(c0, c1):
                pt = ps_t.tile([128, 512], f32, tag="psumT", name="psumT")
                nc.tensor.matmul(pt, ka[hh][:, c * 128:(c + 1) * 128],
                                 qa[hh][:, qs], start=True, stop=False)
                nc.tensor.matmul(pt, kbc[hh][:, c * 128:(c + 1) * 128],
                                 qbc[hh][:, qs], start=False, stop=True)
                g_sb = gs_pool.tile([128, 512], bf16, tag="g", name="g")
                nc.scalar.activation(g_sb, pt, AF.Exp, scale=float(TEMP))
                sg_sb = gs_pool.tile([128, 512], bf16, tag="sg", name="sg")
                nc.scalar.activation(sg_sb, pt, AF.Sign)
                ap_sb = gs_pool.tile([128, 512], bf16, tag="ap", name="ap")
                nc.vector.tensor_scalar(out=ap_sb, in0=g_sb, scalar1=1.0,
                                        scalar2=0.0, op0=OP.subtract,
                                        op1=OP.max)
                st["ap"][c] = ap_sb
                st["sg"][c] = sg_sb
                if c >= 1:
                    emit_av(c - 1, stop=False)

        def p2_end(hh, g):
            SC = sc[hh]
            st = p2s[(hh, g)]
            nc.tensor.matmul(st["av"], va[hh][:, 0, SC - 1, :],
                             st["ap"][SC - 1], start=(SC == 1), stop=False)
            nc.tensor.matmul(st["av"], va[hh][:, 1, SC - 1, :],
                             st["sg"][SC - 1], start=False, stop=True)
            p2s.pop((hh, g))
            # u = av + hs  (hs = [0.5*sum V, 0.5*se, 0.5*se], host-made)
            u_sb = opool.tile([D + 2, 512], f32, tag="u", name="u")
            nc.scalar.activation(u_sb, st["av"], AF.Identity,
                                 bias=hs[hh][:, 0:1])
            cnt_sb = opool.tile([128, 4], f32, tag="cnt", name="cnt")
            for sub in range(4):
                po = ps_x.tile([128, 128], f32, tag="tpose", name="tpose")
                nc.tensor.transpose(po[:, 0:D + 2],
                                    u_sb[:, sub * 128:(sub + 1) * 128],
                                    ident32[0:D + 2, 0:D + 2])
                recip = opool.tile([128, 1], f32, tag="recip", name="recip")
                nc.vector.reciprocal(out=recip, in_=po[:, D:D + 1])
                nc.scalar.copy(out=cnt_sb[:, sub:sub + 1],
                               in_=po[:, D + 1:D + 2])
                o_sb = opool.tile([128, D], f32, tag="osb", name="osb")
                nc.scalar.activation(o_sb, po[:, 0:D], AF.Copy,
                                     scale=recip[:, 0:1])
                lq = g * 512 + sub * 128
                nc.sync.dma_start(out_d[hh][lq:lq + 128, :], o_sb)
            nc.sync.dma_start(cnt_d[hh][g], cnt_sb)

        def ranges(SC):
            base, rem = divmod(SC, 4)
            out, c = [], 0
            for i in range(4):
                w = base + (1 if i < rem else 0)
                out.append((c, c + w))
                c += w
            return out

        units = [(hh, g) for hh in range(HPC) for g in range(QB)]
        for k in range(len(units) + 1):
            cur = units[k] if k < len(units) else None
            prv = units[k - 1] if k >= 1 else None
            if cur is not None and prv is not None:
                rr = ranges(sc[prv[0]])
                for i in range(4):
                    p1_tile(cur[0], cur[1], i)
                    p2_chunks(prv[0], prv[1], rr[i][0], rr[i][1])
                p1_end(*cur)
                p2_end(*prv)
            elif cur is not None:
                for i in range(4):
                    p1_tile(cur[0], cur[1], i)
                p1_end(*cur)
            else:
                p2_chunks(prv[0], prv[1], 0, sc[prv[0]])
                p2_end(*prv)

    nc.compile()
    return nc


_NC_CACHE = {}


def _sc_of(key_lengths_i):
    return tuple(max(1, min(S, int(-(-int(key_lengths_i[n]) // 128))))
                 for n in range(N))


def _get_nc(key_lengths_i):
    key = _sc_of(key_lengths_i)
    if key not in _NC_CACHE:
        _NC_CACHE[key] = _build_bass(key)
    return _NC_CACHE[key]


def _split_hi_lo(x):
    hi = x.astype(_bf16)
    lo = (x.astype(np.float32) - hi.astype(np.float32)).astype(_bf16)
    return hi, lo


def _prep_core(core, queries, keys, values, key_lengths_i):
    """Returns (pairs, in_map) for this core.  pairs = [(n, h)] per slot."""
    sc = _sc_of(key_lengths_i)
    pairs = [(n, core) for n in range(N)]
    im = {}
    for i, (n, h) in enumerate(pairs):
        se = sc[n] * 128
        kl = int(key_lengths_i[n])
        Q = queries[n, :, h, :]             # [L, E]
        K = keys[n, :se, h, :]              # [se, E]
        V = values[n, :se, h, :]            # [se, D]
        qh, ql = _split_hi_lo(Q)
        kh, kl_ = _split_hi_lo(K)
        mask = np.where(np.arange(se) < kl, 0.0, NEG).astype(np.float32)
        qa = np.zeros((128, L), _bf16)
        ka = np.zeros((128, se), _bf16)
        qbc = np.zeros((128, L), _bf16)
        kbc = np.zeros((128, se), _bf16)
        va = np.zeros((2, sc[n], 128, D + 2), _bf16)
        qa[0:E, :] = qh.T
        qa[E, :] = 1.0
        # rows 65..67 stay 0 (t slots, filled on device)
        qa[E + 4:E + 4 + NLO, :] = ql.T[0:NLO]
        ka[0:E, :] = kh.T
        ka[E, :] = mask.astype(_bf16)
        ka[E + 1:E + 4, :] = 1.0
        ka[E + 4:E + 4 + NLO, :] = kl_.T[0:NLO]
        qbc[0:E, :] = qh.T
        qbc[E:2 * E, :] = ql.T
        kbc[0:E, :] = kl_.T
        kbc[E:2 * E, :] = kh.T
        vb = V.astype(_bf16)
        va[0, :, :, 0:D] = vb.reshape(sc[n], 128, D)
        va[0, :, :, D] = 1.0
        va[1, :, :, 0:D] = (0.5 * vb.astype(np.float32)).astype(
            _bf16).reshape(sc[n], 128, D)
        va[1, :, :, D] = 0.5
        va[1, :, :, D + 1] = 0.5
        hsv = np.zeros((D + 2, 1), np.float32)
        hsv[0:D, 0] = 0.5 * vb.astype(np.float32).sum(axis=0)
        hsv[D, 0] = 0.5 * se
        hsv[D + 1, 0] = 0.5 * se
        im[f"qa{i}"] = qa
        im[f"ka{i}"] = ka
        im[f"qbc{i}"] = qbc
        im[f"kbc{i}"] = kbc
        im[f"va{i}"] = va
        im[f"hs{i}"] = hsv
    return pairs, im


def _host_fix_rows(out, rows_by_head, queries, keys, values, key_lengths):
    """Exact fp32 recompute (vectorized per head) of suspect rows."""
    for (n, h), rows in rows_by_head.items():
        if not rows:
            continue
        rows = np.asarray(rows, np.int64)
        kl = int(key_lengths[n])
        Qr = np.asarray(queries[n, rows, h, :], np.float32)      # [R, E]
        K = np.asarray(keys[n, :kl, h, :], np.float32)           # [kl, E]
        V = np.asarray(values[n, :kl, h, :], np.float32)         # [kl, D]
        Sc = Qr @ K.T                                            # [R, kl]
        idx = np.argpartition(-Sc, TOPK - 1, axis=1)[:, :TOPK]   # [R, 32]
        sv = np.take_along_axis(Sc, idx, axis=1)
        w = np.exp(TEMP * (sv - sv.max(axis=1, keepdims=True)))
        o = np.einsum('rk,rkd->rd', w, V[idx]) / w.sum(axis=1)[:, None]
        out[n, rows, h, :] = o


def kernel(queries, keys, values, key_lengths):
    from concourse.bass_utils import run_bass_kernel_spmd

    queries = np.asarray(queries, np.float32)
    keys = np.asarray(keys, np.float32)
    values = np.asarray(values, np.float32)
    key_lengths_i = np.asarray(key_lengths).astype(np.int64)

    in_maps = []
    head_map = []
    for core in range(N_CORES):
        pairs, im = _prep_core(core, queries, keys, values, key_lengths_i)
        head_map.append(pairs)
        in_maps.append(im)

    nc = _get_nc(key_lengths_i)
    res = run_bass_kernel_spmd(nc, in_maps, list(range(N_CORES)))

    out = np.zeros((N, L, H, D), np.float32)
    fix = {}
    for core in range(N_CORES):
        for i, (n, h) in enumerate(head_map[core]):
            out[n, :, h, :] = res.results[core][f"out{i}"].reshape(L, D)
            cnt = res.results[core][f"cnt{i}"].reshape(QB, 128, 4)
            cnt = cnt.transpose(0, 2, 1).reshape(L)
            bad = np.nonzero(cnt != TOPK)[0]
            if len(bad):
                fix.setdefault((n, h), []).extend(int(b) for b in bad)
    if fix:
        _host_fix_rows(out, fix, queries, keys, values, key_lengths_i)
    return out


# revision 37
# speedup vs baseline: 1.0953x; 1.0953x over previous
"""Exact top-k (k=32) attention on 8 Trainium2 NeuronCores.

Head-parallel sharding: core c computes (batch 0, head c) and (batch 1,
head c).  Per-batch key-length truncation: only SC = ceil(kl/128) chunks
of 128 keys are ever touched (the rest can never enter the top-32), so
each core's two head-slots run with different (smaller) S.

Per head, per core:
  Phase 1 (selection): forward scores F[q, s] via a 2-pass bf16-split
    matmul (hi*hi + partial lo*lo in pass A; hi*lo + lo*hi in pass BC;
    ~1e-5 accurate).  Hierarchical exact top-32: per-128-chunk top-8 via
    one DVE max8 each, reading PSUM directly (level 1), then top-32 of
    the <=128 candidates via 4x max8 + 3x match_replace (level 2).
    Cut value t_minus = t - |t|*2^-16 - 1e-37, strictly inside
    (s_33 + eps, s_32); bf16 triple-split of -t_minus is staged into
    rows 65..67 of the qa operand.
  Phase 2 (apply): transposed scores minus t_minus computed by the same
    augmented matmul pair (extra contraction rows carry the mask and -t
    split), giving d'[s, q] = F^T - t_minus in PSUM.  Then
      g = Exp(temp*d')      (ScalarE, bf16)
      S = Sign(d')          (ScalarE, bf16, {-1,+1})
      A' = max(g-1, 0)      (DVE, bf16 2x mode)
    and the weights W = A' + 0.5*(S+1) are applied via two matmul chains
    accumulated into ONE [66, 512] PSUM tile:
      av = va_a^T A' + va_s^T S,   va_a = [V, 1, 0], va_s = 0.5*[V, 1, 1]
    plus a host-precomputed column bias hs = [0.5*sum V, 0.5*se, 0.5*se]
    added during the PSUM->SBUF copy.  Row 64 = Z, row 65 = the EXACT
    selection count (integer); host recomputes rows with count != 32
    (chunk overflow in level 1 or cut-boundary flips; ~1-3% of rows).
  Emission interleaves phase 1 of unit k+1 tile-by-tile with phase 2 of
  unit k chunk-range-by-chunk-range so every engine stays fed.
"""

import numpy as np
import ml_dtypes

N, L, S, H, E, D = 2, 2048, 2048, 8, 64, 64
TOPK = 32
TEMP = 1.0 / np.sqrt(E)
HEADS_PER_CORE = 2
N_CORES = 8
LT = 16          # L tiles of 128
QB = 4           # q groups of 512
NEG = -1e30
NLO = 60         # e-rows of the lo*lo partial correction in pass A

_bf16 = ml_dtypes.bfloat16


def _build_bass(sc):
    """sc: tuple (SC0, SC1) chunk counts (128 keys each) per head-slot."""
    import concourse.mybir as mybir
    from concourse import bacc
    from concourse.tile import TileContext
    from concourse.masks import make_identity

    f32 = mybir.dt.float32
    bf16 = mybir.dt.bfloat16
    OP = mybir.AluOpType
    AF = mybir.ActivationFunctionType

    nc = bacc.Bacc()
    HPC = HEADS_PER_CORE

    qa_d, ka_d, qbc_d, kbc_d, va_d, hs_d, out_d, cnt_d = ([] for _ in
                                                          range(8))
    for hh in range(HPC):
        se = sc[hh] * 128
        qa_d.append(nc.declare_dram_parameter(f"qa{hh}", [128, L], bf16,
                                              isOutput=False))
        ka_d.append(nc.declare_dram_parameter(f"ka{hh}", [128, se], bf16,
                                              isOutput=False))
        qbc_d.append(nc.declare_dram_parameter(f"qbc{hh}", [128, L], bf16,
                                               isOutput=False))
        kbc_d.append(nc.declare_dram_parameter(f"kbc{hh}", [128, se], bf16,
                                               isOutput=False))
        va_d.append(nc.declare_dram_parameter(f"va{hh}", [2, sc[hh], 128,
                                                          D + 2],
                                              bf16, isOutput=False))
        hs_d.append(nc.declare_dram_parameter(f"hs{hh}", [D + 2, 1], f32,
                                              isOutput=False))
        out_d.append(nc.declare_dram_parameter(f"out{hh}", [L, D], f32,
                                               isOutput=True))
        cnt_d.append(nc.declare_dram_parameter(f"cnt{hh}", [QB, 128, 4],
                                               f32, isOutput=True))

    from contextlib import ExitStack
    with TileContext(nc) as tc, ExitStack() as ctx:
        consts = ctx.enter_context(tc.tile_pool(name="consts", bufs=1))
        inpool = ctx.enter_context(tc.tile_pool(name="inputs", bufs=1))
        mpool = ctx.enter_context(tc.tile_pool(name="mbuf", bufs=2))
        small = ctx.enter_context(tc.tile_pool(name="small", bufs=2))
        gs_pool = ctx.enter_context(tc.tile_pool(name="gs", bufs=4))
        opool = ctx.enter_context(tc.tile_pool(name="outbuf", bufs=3))
        ps_f = ctx.enter_context(tc.tile_pool(name="ps_fwd", bufs=3,
                                              space="PSUM"))
        ps_t = ctx.enter_context(tc.tile_pool(name="ps_t", bufs=2,
                                              space="PSUM"))
        ps_av = ctx.enter_context(tc.tile_pool(name="ps_av", bufs=1,
                                               space="PSUM"))
        ps_x = ctx.enter_context(tc.tile_pool(name="ps_x", bufs=1,
                                              space="PSUM"))

        ident = consts.tile([128, 128], bf16)
        make_identity(nc, ident)
        ident32 = consts.tile([128, 128], f32)
        make_identity(nc, ident32)

        # ---- load inputs (head-0 score operands first) ----
        qa, ka, qbc, kbc, va, hs = [], [], [], [], [], []
        for hh in range(HPC):
            se = sc[hh] * 128
            t = inpool.tile([128, se], bf16, tag=f"ka{hh}", name=f"ka{hh}")
            nc.sync.dma_start(t, ka_d[hh][:, :])
            ka.append(t)
            t = inpool.tile([128, L], bf16, tag=f"qa{hh}", name=f"qa{hh}")
            nc.sync.dma_start(t, qa_d[hh][:, :])
            qa.append(t)
            t = inpool.tile([128, se], bf16, tag=f"kbc{hh}", name=f"kbc{hh}")
            nc.sync.dma_start(t, kbc_d[hh][:, :])
            kbc.append(t)
            t = inpool.tile([128, L], bf16, tag=f"qbc{hh}", name=f"qbc{hh}")
            nc.sync.dma_start(t, qbc_d[hh][:, :])
            qbc.append(t)
        for hh in range(HPC):
            t = inpool.tile([128, 2, sc[hh], D + 2], bf16, tag=f"va{hh}",
                            name=f"va{hh}")
            nc.sync.dma_start(t, va_d[hh][:, :, :, :].rearrange(
                "a c p d -> p a c d"))
            va.append(t)
            t = inpool.tile([D + 2, 1], f32, tag=f"hs{hh}", name=f"hs{hh}")
            nc.sync.dma_start(t, hs_d[hh][:, :])
            hs.append(t)

        # per-group phase-1 state: dg (t32 columns), tcols
        p1s = {}
        p2s = {}

        def p1_tile(hh, g, i):
            """fwd scores + level1 + level2 for tile 4g+i of head hh."""
            SC = sc[hh]
            se = SC * 128
            if i == 0:
                p1s[(hh, g)] = {
                    "tcols": small.tile([128, 12], bf16, tag="tcols",
                                        name="tcols"),
                    "dg": opool.tile([128, 4], f32, tag="dg", name="dg"),
                }
            st = p1s[(hh, g)]
            lt = 4 * g + i
            lhsA = qa[hh][:, lt * 128:(lt + 1) * 128]
            lhsBC = qbc[hh][:, lt * 128:(lt + 1) * 128]
            M = mpool.tile([128, SC * 8], f32, tag="M", name="M")
            nblk = (se + 511) // 512
            pfs = []
            # all A-pass matmuls share lhsA (weights stay loaded), then
            # all BC-pass matmuls share lhsBC
            for b in range(nblk):
                cw = min(512, se - 512 * b)
                cs = slice(512 * b, 512 * b + cw)
                pf = ps_f.tile([128, 512], f32, tag="fwd", name="fwd")
                pfs.append(pf)
                nc.tensor.matmul(pf[:, 0:cw], lhsA, ka[hh][:, cs],
                                 start=True, stop=False)
            for b in range(nblk):
                cw = min(512, se - 512 * b)
                cs = slice(512 * b, 512 * b + cw)
                nc.tensor.matmul(pfs[b][:, 0:cw], lhsBC, kbc[hh][:, cs],
                                 start=False, stop=True)
                for c in range(cw // 128):
                    cc = 4 * b + c
                    nc.vector.max(out=M[:, 8 * cc:8 * cc + 8],
                                  in_=pfs[b][:, 128 * c:128 * c + 128])
            m32 = small.tile([128, 32], f32, tag="m32", name="m32")
            for r in range(4):
                nc.vector.max(out=m32[:, 8 * r:8 * r + 8], in_=M)
                if r < 3:
                    nc.vector.match_replace(
                        out=M, in_to_replace=m32[:, 8 * r:8 * r + 8],
                        in_values=M, imm_value=NEG)
            nc.vector.tensor_copy(st["dg"][:, i:i + 1], m32[:, 31:32])

        def p1_end(hh, g):
            """batched t-ops + staging of the -t_minus bf16 triple-split."""
            st = p1s.pop((hh, g))
            tcols, dg = st["tcols"], st["dg"]
            aco = small.tile([128, 12], f32, tag="aco", name="aco")
            nc.scalar.activation(aco[:, 0:4], dg, AF.Abs,
                                 scale=float(2.0 ** -16))
            nc.vector.scalar_tensor_tensor(
                out=aco[:, 4:8], in0=aco[:, 0:4], scalar=1e-37,
                in1=dg, op0=OP.add, op1=OP.subtract)
            nc.vector.tensor_copy(tcols[:, 0:4], aco[:, 4:8])
            nc.vector.tensor_tensor(out=aco[:, 8:12], in0=aco[:, 4:8],
                                    in1=tcols[:, 0:4], op=OP.subtract)
            nc.vector.tensor_copy(tcols[:, 4:8], aco[:, 8:12])
            nc.vector.tensor_tensor(out=aco[:, 0:4], in0=aco[:, 8:12],
                                    in1=tcols[:, 4:8], op=OP.subtract)
            nc.vector.tensor_copy(tcols[:, 8:12], aco[:, 0:4])
            pt = ps_x.tile([128, 128], bf16, tag="tposeb", name="tposeb")
            nc.tensor.transpose(pt[0:12, :], tcols, ident)
            stage = small.tile([12, 128], bf16, tag="stage12", name="stage12")
            nc.scalar.copy(out=stage, in_=pt[0:12, :])
            for j in range(3):
                nc.sync.dma_start(
                    qa[hh][65 + j:66 + j, g * 512:(g + 1) * 512].rearrange(
                        "p (t q) -> p t q", t=4),
                    stage[4 * j:4 * (j + 1), :])

        def p2_chunks(hh, g, c0, c1):
            """d' + masks + AV for chunks [c0, c1) of unit (hh, g)."""
            SC = sc[hh]
            qs = slice(g * 512, (g + 1) * 512)
            if c0 == 0:
                p2s[(hh, g)] = {
                    "av": ps_av.tile([D + 2, 512], f32, tag="av", name="av"),
                    "ap": [None] * SC, "sg": [None] * SC,
                }
            st = p2s[(hh, g)]

            def emit_av(c, stop):
                nc.tensor.matmul(st["av"], va[hh][:, 0, c, :], st["ap"][c],
                                 start=(c == 0), stop=False)
                nc.tensor.matmul(st["av"], va[hh][:, 1, c, :], st["sg"][c],
                                 start=False, stop=stop)

            for c in range(c0, c1):
                pt = ps_t.tile([128, 512], f32, tag="psumT", name="psumT")
                nc.tensor.matmul(pt, ka[hh][:, c * 128:(c + 1) * 128],
                                 qa[hh][:, qs], start=True, stop=False)
                nc.tensor.matmul(pt, kbc[hh][:, c * 128:(c + 1) * 128],
                                 qbc[hh][:, qs], start=False, stop=True)
                sg_sb = gs_pool.tile([128, 512], bf16, tag="sg", name="sg")
                nc.scalar.activation(sg_sb, pt, AF.Sign)
                g_sb = gs_pool.tile([128, 512], bf16, tag="g", name="g")
                nc.scalar.activation(g_sb, pt, AF.Exp, scale=float(TEMP))
                ap_sb = gs_pool.tile([128, 512], bf16, tag="ap", name="ap")
                nc.vector.tensor_scalar(out=ap_sb, in0=g_sb, scalar1=1.0,
                                        scalar2=0.0, op0=OP.subtract,
                                        op1=OP.max)
                st["ap"][c] = ap_sb
                st["sg"][c] = sg_sb
                if c >= 2:
                    emit_av(c - 2, stop=False)

        def p2_end(hh, g):
            SC = sc[hh]
            st = p2s[(hh, g)]
            if SC >= 2:
                nc.tensor.matmul(st["av"], va[hh][:, 0, SC - 2, :],
                                 st["ap"][SC - 2], start=(SC == 2),
                                 stop=False)
                nc.tensor.matmul(st["av"], va[hh][:, 1, SC - 2, :],
                                 st["sg"][SC - 2], start=False, stop=False)
            nc.tensor.matmul(st["av"], va[hh][:, 0, SC - 1, :],
                             st["ap"][SC - 1], start=(SC == 1), stop=False)
            nc.tensor.matmul(st["av"], va[hh][:, 1, SC - 1, :],
                             st["sg"][SC - 1], start=False, stop=True)
            p2s.pop((hh, g))
            # u = av + hs  (hs = [0.5*sum V, 0.5*se, 0.5*se], host-made)
            u_sb = opool.tile([D + 2, 512], f32, tag="u", name="u")
            nc.scalar.activation(u_sb, st["av"], AF.Identity,
                                 bias=hs[hh][:, 0:1])
            cnt_sb = opool.tile([128, 4], f32, tag="cnt", name="cnt")
            for sub in range(4):
                po = ps_x.tile([128, 128], f32, tag="tpose", name="tpose")
                nc.tensor.transpose(po[:, 0:D + 2],
                                    u_sb[:, sub * 128:(sub + 1) * 128],
                                    ident32[0:D + 2, 0:D + 2])
                recip = opool.tile([128, 1], f32, tag="recip", name="recip")
                nc.vector.reciprocal(out=recip, in_=po[:, D:D + 1])
                nc.scalar.copy(out=cnt_sb[:, sub:sub + 1],
                               in_=po[:, D + 1:D + 2])
                o_sb = opool.tile([128, D], f32, tag="osb", name="osb")
                nc.scalar.activation(o_sb, po[:, 0:D], AF.Copy,
                                     scale=recip[:, 0:1])
                lq = g * 512 + sub * 128
                nc.sync.dma_start(out_d[hh][lq:lq + 128, :], o_sb)
            nc.sync.dma_start(cnt_d[hh][g], cnt_sb)

        def ranges(SC):
            base, rem = divmod(SC, 4)
            out, c = [], 0
            for i in range(4):
                w = base + (1 if i < rem else 0)
                out.append((c, c + w))
                c += w
            return out

        units = [(hh, g) for hh in range(HPC) for g in range(QB)]
        for k in range(len(units) + 1):
            cur = units[k] if k < len(units) else None
            prv = units[k - 1] if k >= 1 else None
            if cur is not None and prv is not None:
                rr = ranges(sc[prv[0]])
                for i in range(4):
                    p1_tile(cur[0], cur[1], i)
                    p2_chunks(prv[0], prv[1], rr[i][0], rr[i][1])
                p1_end(*cur)
                p2_end(*prv)
            elif cur is not None:
                for i in range(4):
                    p1_tile(cur[0], cur[1], i)
                p1_end(*cur)
            else:
                p2_chunks(prv[0], prv[1], 0, sc[prv[0]])
                p2_end(*prv)

    nc.compile()
    return nc


_NC_CACHE = {}


def _sc_of(key_lengths_i):
    return tuple(max(1, min(S, int(-(-int(key_lengths_i[n]) // 128))))
                 for n in range(N))


def _get_nc(key_lengths_i):
    key = _sc_of(key_lengths_i)
    if key not in _NC_CACHE:
        _NC_CACHE[key] = _build_bass(key)
    return _NC_CACHE[key]


def _split_hi_lo(x):
    hi = x.astype(_bf16)
    lo = (x.astype(np.float32) - hi.astype(np.float32)).astype(_bf16)
    return hi, lo


def _prep_core(core, queries, keys, values, key_lengths_i):
    """Returns (pairs, in_map) for this core.  pairs = [(n, h)] per slot."""
    sc = _sc_of(key_lengths_i)
    pairs = [(n, core) for n in range(N)]
    im = {}
    for i, (n, h) in enumerate(pairs):
        se = sc[n] * 128
        kl = int(key_lengths_i[n])
        Q = queries[n, :, h, :]             # [L, E]
        K = keys[n, :se, h, :]              # [se, E]
        V = values[n, :se, h, :]            # [se, D]
        qh, ql = _split_hi_lo(Q)
        kh, kl_ = _split_hi_lo(K)
        mask = np.where(np.arange(se) < kl, 0.0, NEG).astype(np.float32)
        qa = np.zeros((128, L), _bf16)
        ka = np.zeros((128, se), _bf16)
        qbc = np.zeros((128, L), _bf16)
        kbc = np.zeros((128, se), _bf16)
        va = np.zeros((2, sc[n], 128, D + 2), _bf16)
        qa[0:E, :] = qh.T
        qa[E, :] = 1.0
        # rows 65..67 stay 0 (t slots, filled on device)
        qa[E + 4:E + 4 + NLO, :] = ql.T[0:NLO]
        ka[0:E, :] = kh.T
        ka[E, :] = mask.astype(_bf16)
        ka[E + 1:E + 4, :] = 1.0
        ka[E + 4:E + 4 + NLO, :] = kl_.T[0:NLO]
        qbc[0:E, :] = qh.T
        qbc[E:2 * E, :] = ql.T
        kbc[0:E, :] = kl_.T
        kbc[E:2 * E, :] = kh.T
        vb = V.astype(_bf16)
        va[0, :, :, 0:D] = vb.reshape(sc[n], 128, D)
        va[0, :, :, D] = 1.0
        va[1, :, :, 0:D] = (0.5 * vb.astype(np.float32)).astype(
            _bf16).reshape(sc[n], 128, D)
        va[1, :, :, D] = 0.5
        va[1, :, :, D + 1] = 0.5
        hsv = np.zeros((D + 2, 1), np.float32)
        hsv[0:D, 0] = 0.5 * vb.astype(np.float32).sum(axis=0)
        hsv[D, 0] = 0.5 * se
        hsv[D + 1, 0] = 0.5 * se
        im[f"qa{i}"] = qa
        im[f"ka{i}"] = ka
        im[f"qbc{i}"] = qbc
        im[f"kbc{i}"] = kbc
        im[f"va{i}"] = va
        im[f"hs{i}"] = hsv
    return pairs, im


def _host_fix_rows(out, rows_by_head, queries, keys, values, key_lengths):
    """Exact fp32 recompute (vectorized per head) of suspect rows."""
    for (n, h), rows in rows_by_head.items():
        if not rows:
            continue
        rows = np.asarray(rows, np.int64)
        kl = int(key_lengths[n])
        Qr = np.asarray(queries[n, rows, h, :], np.float32)      # [R, E]
        K = np.asarray(keys[n, :kl, h, :], np.float32)           # [kl, E]
        V = np.asarray(values[n, :kl, h, :], np.float32)         # [kl, D]
        Sc = Qr @ K.T                                            # [R, kl]
        idx = np.argpartition(-Sc, TOPK - 1, axis=1)[:, :TOPK]   # [R, 32]
        sv = np.take_along_axis(Sc, idx, axis=1)
        w = np.exp(TEMP * (sv - sv.max(axis=1, keepdims=True)))
        o = np.einsum('rk,rkd->rd', w, V[idx]) / w.sum(axis=1)[:, None]
        out[n, rows, h, :] = o


def kernel(queries, keys, values, key_lengths):
    from concourse.bass_utils import run_bass_kernel_spmd

    queries = np.asarray(queries, np.float32)
    keys = np.asarray(keys, np.float32)
    values = np.asarray(values, np.float32)
    key_lengths_i = np.asarray(key_lengths).astype(np.int64)

    in_maps = []
    head_map = []
    for core in range(N_CORES):
        pairs, im = _prep_core(core, queries, keys, values, key_lengths_i)
        head_map.append(pairs)
        in_maps.append(im)

    nc = _get_nc(key_lengths_i)
    res = run_bass_kernel_spmd(nc, in_maps, list(range(N_CORES)))

    out = np.zeros((N, L, H, D), np.float32)
    fix = {}
    for core in range(N_CORES):
        for i, (n, h) in enumerate(head_map[core]):
            out[n, :, h, :] = res.results[core][f"out{i}"].reshape(L, D)
            cnt = res.results[core][f"cnt{i}"].reshape(QB, 128, 4)
            cnt = cnt.transpose(0, 2, 1).reshape(L)
            bad = np.nonzero(cnt != TOPK)[0]
            if len(bad):
                fix.setdefault((n, h), []).extend(int(b) for b in bad)
    if fix:
        _host_fix_rows(out, fix, queries, keys, values, key_lengths_i)
    return out
